# revision 7
# baseline (speedup 1.0000x reference)
"""GCMCGraphConv forward on 8 trn2 NeuronCores (Bass/Tile).

reference:
    rf  = review_feat @ w_review.T                      [E, F]
    msg = (x[src] + weight[src] + rf) * ci[src]         [E, F]
    h   = segment_sum(msg, dst, N)                      [N, F]
    out = h * ci

Strategy (dst-owner sharding, edge-parallel within a core):
  - Core c owns nodes [c*NPC, (c+1)*NPC). Host routes every edge to the
    owner of its dst, groups the core's edges by 128-node block of dst,
    and pads each block's edge list to whole 128-edge tiles.  Tile
    counts per block are maxed across cores so all 8 cores run one SPMD
    program.
  - Device, per 128-edge tile:
      * one indirect DMA gathers 128 rows (528B) of a host-packed node
        table T = [x | weight | ci | pad] at the tile's src ids
      * DVE builds a ci-scaled one-hot:  S[e,n] = ci[src_e]*(dstloc_e==n)
      * PE accumulates into PSUM over the block's tiles:
          psA[n,0:64] += S^T @ x_rows ; psA += S^T @ w_rows
          psB[k,n]    += rfeat_tile(stationary) @ S       (= B^T)
  - Per block (once):  psA += B @ w_review^T
    via matmul(lhsT=B^T, rhs=w_review^T), then out_block = psA * ci[dst].
    (w_review commutes with the segment sum, so it is applied once per
     128-node block instead of once per edge.)

Host does index math / layout only (routing, padding, permutation,
concatenation); all float compute (gathers, messages, sums, matmuls,
scaling) runs on device.
"""

import os
import numpy as np
from contextlib import ExitStack

import concourse.bass as bass
import concourse.tile as tile
from concourse import bacc, mybir
from concourse.bass_utils import run_bass_kernel_spmd

P = 128
TBL_W = 132          # table row: 64 x | 64 w | 1 ci | 3 pad  (528B)
CHUNK = 16           # edge-tiles per rfeat DMA chunk
MBATCH = 256         # tiles per offs/dls metadata DMA
PAD_DL = 16000.0     # dstloc sentinel for pad edges -> one-hot column is 0

N_NODES = 150000
N_EDGES = 1250000
FEAT = 64
N_CORES = 8

# offsets handed to the indirect gather as a strided [P,1] column of the
# batched metadata tile (0) or copied to a dense [P,1] tile first (1).
OFF_COPY = bool(int(os.environ.get("GCMC_OFFCOPY", "0")))


# --------------------------------------------------------------- host prep

def host_prep(x, weight, w_review, review_feat, ci, src, dst, n_cores):
    """Route edges to dst-owner cores, build per-core DMA-friendly arrays.

    Index math and layout only -- no feature arithmetic happens here.
    """
    N, F = x.shape
    NPC = N // n_cores
    K = (NPC + P - 1) // P
    owner = dst // NPC

    per_core = []
    counts = np.zeros((n_cores, K), np.int64)
    for c in range(n_cores):
        sel = np.nonzero(owner == c)[0]
        blk = (dst[sel] - c * NPC) >> 7
        order = np.argsort(blk, kind="stable")
        per_core.append((sel[order], blk[order]))
        counts[c] = np.bincount(blk, minlength=K)

    nt_k = np.maximum(1, -(-counts.max(axis=0) // P))
    NT = int(nt_k.sum())
    NT16 = -(-NT // CHUNK) * CHUNK
    nt_k[K - 1] += NT16 - NT
    NT = NT16
    t0 = np.zeros(K + 1, np.int64)
    t0[1:] = np.cumsum(nt_k)

    table = np.zeros((N, TBL_W), np.float32)
    table[:, 0:F] = x
    table[:, F:2 * F] = weight
    table[:, 2 * F] = ci[:, 0]
    wrT = np.ascontiguousarray(w_review.T)               # [k, f]

    # slot -> rfeat DRAM row permutation (2 rows per 512B DMA line)
    slot_ids = np.arange(NT * P)
    t_of = slot_ids // P
    p_of = slot_ids % P
    tl = t_of % CHUNK
    rf_row = (((t_of // CHUNK) * 8 + tl // 2) * P + p_of) * 2 + (tl % 2)

    in_maps = []
    for c in range(n_cores):
        eids, blks = per_core[c]
        cnt = counts[c]
        base = np.concatenate([[0], np.cumsum(cnt)[:-1]])
        slotpos = t0[blks] * P + (np.arange(len(eids)) - base[blks])

        slots_src = np.zeros(NT * P, np.int32)
        slots_dl = np.full(NT * P, PAD_DL, np.float32)
        slots_src[slotpos] = src[eids]
        slots_dl[slotpos] = (dst[eids] - c * NPC - blks * P).astype(np.float32)

        rf = np.zeros((NT * P, F), np.float32)
        rf[rf_row[slotpos]] = review_feat[eids]

        nodes = c * NPC + np.arange(K * P)
        cic = np.zeros(K * P, np.float32)
        v = nodes < (c + 1) * NPC
        cic[v] = ci[nodes[v], 0]

        in_maps.append({
            "table": table,
            "wrT": wrT,
            "offs": np.ascontiguousarray(slots_src.reshape(NT, P).T),
            "dls": np.ascontiguousarray(slots_dl.reshape(NT, P).T),
            "rfs": rf,
            "cic": np.ascontiguousarray(cic.reshape(K, P).T),
        })

    meta = dict(N=N, F=F, NPC=NPC, K=K, NT=NT, n_cores=n_cores,
                nt_k=nt_k.tolist())
    return in_maps, meta


# ------------------------------------------------------------- bass program

def build_program(meta, reps=1):
    """Build the SPMD program.  reps>1 wraps the whole kernel in a hardware
    loop that re-executes it (idempotently) for wall-clock timing."""
    N = meta["N"]; F = meta["F"]; NPC = meta["NPC"]; K = meta["K"]
    NT = meta["NT"]; nt_k = meta["nt_k"]; n_cores = meta["n_cores"]
    F2 = 2 * F
    dt = mybir.dt

    t0 = np.zeros(K + 1, np.int64)
    t0[1:] = np.cumsum(nt_k)
    tile_block = np.repeat(np.arange(K), nt_k)

    nc = bacc.Bacc("TRN2", target_bir_lowering=False, debug=False,
                   enable_asserts=False, num_devices=n_cores)

    table = nc.dram_tensor("table", [N, TBL_W], dt.float32,
                           kind="ExternalInput").ap()
    wrT = nc.dram_tensor("wrT", [F, F], dt.float32, kind="ExternalInput").ap()
    offs = nc.dram_tensor("offs", [P, NT], dt.int32, kind="ExternalInput").ap()
    dls = nc.dram_tensor("dls", [P, NT], dt.float32,
                         kind="ExternalInput").ap()
    rfs = nc.dram_tensor("rfs", [NT * P, F], dt.float32,
                         kind="ExternalInput").ap()
    cic = nc.dram_tensor("cic", [P, K], dt.float32, kind="ExternalInput").ap()
    out = nc.dram_tensor("out", [NPC, F], dt.float32,
                         kind="ExternalOutput").ap()

    rf_view = rfs.rearrange("(c j p h) f -> c p j h f", j=8, p=P, h=2)

    with tile.TileContext(nc) as tc, ExitStack() as ctx:
        consts = ctx.enter_context(tc.tile_pool(name="consts", bufs=1))
        mpool = ctx.enter_context(tc.tile_pool(name="meta", bufs=2))
        gpool = ctx.enter_context(tc.tile_pool(name="gather", bufs=24))
        ofpool = ctx.enter_context(tc.tile_pool(name="ofp", bufs=24))
        rfpool = ctx.enter_context(tc.tile_pool(name="rfeat", bufs=4))
        ohpool = ctx.enter_context(tc.tile_pool(name="onehot", bufs=8))
        opool = ctx.enter_context(tc.tile_pool(name="outs", bufs=4))
        btpool = ctx.enter_context(tc.tile_pool(name="btile", bufs=3))
        psa = ctx.enter_context(tc.tile_pool(name="psa", bufs=3, space="PSUM"))
        psb = ctx.enter_context(tc.tile_pool(name="psb", bufs=3, space="PSUM"))

        iota_i = consts.tile([P, P], dt.int32, tag="iota_i")
        nc.gpsimd.iota(iota_i[:], pattern=[[1, P]], base=0,
                       channel_multiplier=0)
        iota_f = consts.tile([P, P], dt.float32, tag="iota_f")
        nc.vector.tensor_copy(iota_f[:], iota_i[:])
        wrT_sb = consts.tile([F, F], dt.float32, tag="wrT")
        nc.sync.dma_start(out=wrT_sb[:], in_=wrT[:])
        cic_sb = consts.tile([P, K], dt.float32, tag="cic")
        nc.sync.dma_start(out=cic_sb[:], in_=cic[:])

        def body(iv=None):
            offs_sb = dls_sb = rfc = None
            psA = psB = None
            for t in range(NT):
                k = int(tile_block[t])
                first = (t == t0[k])
                last = (t == t0[k + 1] - 1)
                mb = t % MBATCH
                if mb == 0:
                    w = min(MBATCH, NT - t)
                    offs_sb = mpool.tile([P, MBATCH], dt.int32, tag="offs")
                    nc.sync.dma_start(out=offs_sb[:, :w],
                                      in_=offs[:, t:t + w])
                    dls_sb = mpool.tile([P, MBATCH], dt.float32, tag="dls")
                    nc.sync.dma_start(out=dls_sb[:, :w], in_=dls[:, t:t + w])
                if t % CHUNK == 0:
                    rfc = rfpool.tile([P, CHUNK * F], dt.float32, tag="rfc")
                    nc.sync.dma_start(out=rfc[:], in_=rf_view[t // CHUNK])

                if OFF_COPY:
                    oc = ofpool.tile([P, 1], dt.int32, tag="oc")
                    nc.vector.tensor_copy(oc[:], offs_sb[:, mb:mb + 1])
                    off_ap = oc[:]
                else:
                    off_ap = offs_sb[:, mb:mb + 1]
                g = gpool.tile([P, TBL_W], dt.float32, tag="g")
                nc.gpsimd.indirect_dma_start(
                    out=g[:], out_offset=None, in_=table[:],
                    in_offset=bass.IndirectOffsetOnAxis(ap=off_ap, axis=0))

                if first:
                    psA = psa.tile([P, F], dt.float32, tag="psA")
                    psB = psb.tile([F, P], dt.float32, tag="psB")

                oh = ohpool.tile([P, P], dt.float32, tag="oh")
                # S = (iota == dstloc) * ci_src
                nc.vector.tensor_scalar(
                    out=oh[:], in0=iota_f[:],
                    scalar1=dls_sb[:, mb:mb + 1],
                    scalar2=g[:, F2:F2 + 1],
                    op0=mybir.AluOpType.is_equal, op1=mybir.AluOpType.mult)

                nc.tensor.matmul(psA[:], lhsT=oh[:], rhs=g[:, 0:F],
                                 start=first, stop=False)
                nc.tensor.matmul(psA[:], lhsT=oh[:], rhs=g[:, F:F2],
                                 start=False, stop=False)
                tl = t % CHUNK
                nc.tensor.matmul(psB[:], lhsT=rfc[:, tl * F: tl * F + F],
                                 rhs=oh[:], start=first, stop=last)

                if last:
                    bt = btpool.tile([F, P], dt.float32, tag="bt")
                    nc.scalar.copy(bt[:], psB[:])
                    nc.tensor.matmul(psA[:], lhsT=bt[:], rhs=wrT_sb[:],
                                     start=False, stop=True)
                    o2 = opool.tile([P, F], dt.float32, tag="o2")
                    nc.vector.tensor_scalar_mul(o2[:], psA[:],
                                                cic_sb[:, k: k + 1])
                    rows = min(P, NPC - k * P)
                    nc.sync.dma_start(out=out[k * P: k * P + rows, :],
                                      in_=o2[:rows, :])

        if reps == 1:
            body()
        else:
            with tc.For_i(0, reps, 1) as iv:
                body(iv)

    nc.compile()
    return nc


# ------------------------------------------------------------------ driver

_CACHE = {}


def _get_program(meta, reps=1):
    key = (meta["N"], meta["F"], meta["NPC"], meta["K"], meta["NT"],
           meta["n_cores"], tuple(meta["nt_k"]), reps)
    if key not in _CACHE:
        _CACHE[key] = build_program(meta, reps=reps)
    return _CACHE[key]


def run(inputs, n_cores=N_CORES, trace=False, reps=1):
    in_maps, meta = host_prep(
        inputs["x"], inputs["weight"], inputs["w_review"],
        inputs["review_feat"], inputs["ci"], inputs["src"], inputs["dst"],
        n_cores)
    nc = _get_program(meta, reps=reps)
    res = run_bass_kernel_spmd(nc, in_maps, list(range(n_cores)), trace=trace)
    outp = np.concatenate([res.results[c]["out"] for c in range(n_cores)],
                          axis=0)
    return outp, res


def kernel(**inputs) -> np.ndarray:
    inputs = {k: np.asarray(v) for k, v in inputs.items()}
    outp, _ = run(inputs, n_cores=N_CORES)
    return outp


# revision 9
# speedup vs baseline: 4.5826x; 4.5826x over previous
"""GCMCGraphConv forward on 8 trn2 NeuronCores (Bass/Tile).

reference:
    rf  = review_feat @ w_review.T                      [E, F]
    msg = (x[src] + weight[src] + rf) * ci[src]         [E, F]
    h   = segment_sum(msg, dst, N)                      [N, F]
    out = h * ci

Strategy (dst-owner sharding, edge-parallel within a core):
  - Core c owns nodes [c*NPC, (c+1)*NPC). Host routes every edge to the
    owner of its dst, groups the core's edges by 128-node block of dst,
    and pads each block's edge list to whole 128-edge tiles.  Tile
    counts per block are maxed across cores so all 8 cores run one SPMD
    program.
  - Device, per 128-edge tile:
      * one indirect DMA gathers 128 rows (528B) of a host-packed node
        table T = [x | weight | ci | pad] at the tile's src ids
      * DVE builds a ci-scaled one-hot:  S[e,n] = ci[src_e]*(dstloc_e==n)
      * PE accumulates into PSUM over the block's tiles:
          psA[n,0:64] += S^T @ x_rows ; psA += S^T @ w_rows
          psB[k,n]    += rfeat_tile(stationary) @ S       (= B^T)
  - Per block (once):  psA += B @ w_review^T
    via matmul(lhsT=B^T, rhs=w_review^T), then out_block = psA * ci[dst].
    (w_review commutes with the segment sum, so it is applied once per
     128-node block instead of once per edge.)

Host does index math / layout only (routing, padding, permutation,
concatenation); all float compute (gathers, messages, sums, matmuls,
scaling) runs on device.
"""

import os
import numpy as np
from contextlib import ExitStack

import concourse.bass as bass
import concourse.tile as tile
from concourse import bacc, mybir
from concourse.bass_utils import run_bass_kernel_spmd

P = 128
TBL_W = 132          # table row: 64 x | 64 w | 1 ci | 3 pad  (528B)
CHUNK = 16           # edge-tiles per rfeat DMA chunk
MBATCH = 256         # tiles per offs/dls metadata DMA
PAD_DL = 16000.0     # dstloc sentinel for pad edges -> one-hot column is 0

N_NODES = 150000
N_EDGES = 1250000
FEAT = 64
N_CORES = 8

# offsets handed to the indirect gather as a strided [P,1] column of the
# batched metadata tile (0) or copied to a dense [P,1] tile first (1).
OFF_COPY = bool(int(os.environ.get("GCMC_OFFCOPY", "0")))
# timing experiment: replace indirect gathers with contiguous loads of the
# same size (removes all Pool-engine work; output is wrong)
NO_GATHER = bool(int(os.environ.get("GCMC_NOGATHER", "0")))


# --------------------------------------------------------------- host prep

def host_prep(x, weight, w_review, review_feat, ci, src, dst, n_cores):
    """Route edges to dst-owner cores, build per-core DMA-friendly arrays.

    Index math and layout only -- no feature arithmetic happens here.
    """
    N, F = x.shape
    NPC = N // n_cores
    K = (NPC + P - 1) // P
    owner = dst // NPC

    per_core = []
    counts = np.zeros((n_cores, K), np.int64)
    for c in range(n_cores):
        sel = np.nonzero(owner == c)[0]
        blk = (dst[sel] - c * NPC) >> 7
        order = np.argsort(blk, kind="stable")
        per_core.append((sel[order], blk[order]))
        counts[c] = np.bincount(blk, minlength=K)

    nt_k = np.maximum(1, -(-counts.max(axis=0) // P))
    NT = int(nt_k.sum())
    NT16 = -(-NT // CHUNK) * CHUNK
    nt_k[K - 1] += NT16 - NT
    NT = NT16
    t0 = np.zeros(K + 1, np.int64)
    t0[1:] = np.cumsum(nt_k)

    table = np.zeros((N, TBL_W), np.float32)
    table[:, 0:F] = x
    table[:, F:2 * F] = weight
    table[:, 2 * F] = ci[:, 0]
    wrT = np.ascontiguousarray(w_review.T)               # [k, f]

    # slot -> rfeat DRAM row permutation (2 rows per 512B DMA line)
    slot_ids = np.arange(NT * P)
    t_of = slot_ids // P
    p_of = slot_ids % P
    tl = t_of % CHUNK
    rf_row = (((t_of // CHUNK) * 8 + tl // 2) * P + p_of) * 2 + (tl % 2)

    in_maps = []
    for c in range(n_cores):
        eids, blks = per_core[c]
        cnt = counts[c]
        base = np.concatenate([[0], np.cumsum(cnt)[:-1]])
        slotpos = t0[blks] * P + (np.arange(len(eids)) - base[blks])

        slots_src = np.zeros(NT * P, np.int32)
        slots_dl = np.full(NT * P, PAD_DL, np.float32)
        slots_src[slotpos] = src[eids]
        slots_dl[slotpos] = (dst[eids] - c * NPC - blks * P).astype(np.float32)

        rf = np.zeros((NT * P, F), np.float32)
        rf[rf_row[slotpos]] = review_feat[eids]

        nodes = c * NPC + np.arange(K * P)
        cic = np.zeros(K * P, np.float32)
        v = nodes < (c + 1) * NPC
        cic[v] = ci[nodes[v], 0]

        in_maps.append({
            "table": table,
            "wrT": wrT,
            "offs": np.ascontiguousarray(slots_src.reshape(NT, P).T),
            "dls": np.ascontiguousarray(slots_dl.reshape(NT, P).T),
            "rfs": rf,
            "cic": np.ascontiguousarray(cic.reshape(K, P).T),
        })

    meta = dict(N=N, F=F, NPC=NPC, K=K, NT=NT, n_cores=n_cores,
                nt_k=nt_k.tolist())
    return in_maps, meta


# ------------------------------------------------------------- bass program

def build_program(meta, reps=1):
    """Build the SPMD program.  reps>1 wraps the whole kernel in a hardware
    loop that re-executes it (idempotently) for wall-clock timing."""
    N = meta["N"]; F = meta["F"]; NPC = meta["NPC"]; K = meta["K"]
    NT = meta["NT"]; nt_k = meta["nt_k"]; n_cores = meta["n_cores"]
    F2 = 2 * F
    dt = mybir.dt

    t0 = np.zeros(K + 1, np.int64)
    t0[1:] = np.cumsum(nt_k)
    tile_block = np.repeat(np.arange(K), nt_k)

    nc = bacc.Bacc("TRN2", target_bir_lowering=False, debug=False,
                   enable_asserts=False, num_devices=n_cores)

    table = nc.dram_tensor("table", [N, TBL_W], dt.float32,
                           kind="ExternalInput").ap()
    wrT = nc.dram_tensor("wrT", [F, F], dt.float32, kind="ExternalInput").ap()
    offs = nc.dram_tensor("offs", [P, NT], dt.int32, kind="ExternalInput").ap()
    dls = nc.dram_tensor("dls", [P, NT], dt.float32,
                         kind="ExternalInput").ap()
    rfs = nc.dram_tensor("rfs", [NT * P, F], dt.float32,
                         kind="ExternalInput").ap()
    cic = nc.dram_tensor("cic", [P, K], dt.float32, kind="ExternalInput").ap()
    out = nc.dram_tensor("out", [NPC, F], dt.float32,
                         kind="ExternalOutput").ap()

    rf_view = rfs.rearrange("(c j p h) f -> c p j h f", j=8, p=P, h=2)

    with tile.TileContext(nc) as tc, ExitStack() as ctx:
        consts = ctx.enter_context(tc.tile_pool(name="consts", bufs=1))
        mpool = ctx.enter_context(tc.tile_pool(name="meta", bufs=2))
        gpool = ctx.enter_context(tc.tile_pool(name="gather", bufs=24))
        ofpool = ctx.enter_context(tc.tile_pool(name="ofp", bufs=24))
        rfpool = ctx.enter_context(tc.tile_pool(name="rfeat", bufs=4))
        ohpool = ctx.enter_context(tc.tile_pool(name="onehot", bufs=8))
        opool = ctx.enter_context(tc.tile_pool(name="outs", bufs=4))
        btpool = ctx.enter_context(tc.tile_pool(name="btile", bufs=3))
        psa = ctx.enter_context(tc.tile_pool(name="psa", bufs=3, space="PSUM"))
        psb = ctx.enter_context(tc.tile_pool(name="psb", bufs=3, space="PSUM"))

        iota_i = consts.tile([P, P], dt.int32, tag="iota_i")
        nc.gpsimd.iota(iota_i[:], pattern=[[1, P]], base=0,
                       channel_multiplier=0)
        iota_f = consts.tile([P, P], dt.float32, tag="iota_f")
        nc.vector.tensor_copy(iota_f[:], iota_i[:])
        wrT_sb = consts.tile([F, F], dt.float32, tag="wrT")
        nc.sync.dma_start(out=wrT_sb[:], in_=wrT[:])
        cic_sb = consts.tile([P, K], dt.float32, tag="cic")
        nc.sync.dma_start(out=cic_sb[:], in_=cic[:])

        def body(iv=None):
            offs_sb = dls_sb = rfc = None
            psA = psB = None
            for t in range(NT):
                k = int(tile_block[t])
                first = (t == t0[k])
                last = (t == t0[k + 1] - 1)
                mb = t % MBATCH
                if mb == 0:
                    w = min(MBATCH, NT - t)
                    offs_sb = mpool.tile([P, MBATCH], dt.int32, tag="offs")
                    nc.sync.dma_start(out=offs_sb[:, :w],
                                      in_=offs[:, t:t + w])
                    dls_sb = mpool.tile([P, MBATCH], dt.float32, tag="dls")
                    nc.sync.dma_start(out=dls_sb[:, :w], in_=dls[:, t:t + w])
                if t % CHUNK == 0:
                    rfc = rfpool.tile([P, CHUNK * F], dt.float32, tag="rfc")
                    nc.sync.dma_start(out=rfc[:], in_=rf_view[t // CHUNK])

                if OFF_COPY:
                    oc = ofpool.tile([P, 1], dt.int32, tag="oc")
                    nc.vector.tensor_copy(oc[:], offs_sb[:, mb:mb + 1])
                    off_ap = oc[:]
                else:
                    off_ap = offs_sb[:, mb:mb + 1]
                g = gpool.tile([P, TBL_W], dt.float32, tag="g")
                if NO_GATHER:
                    nc.sync.dma_start(
                        out=g[:], in_=table[(t % 1000) * P:(t % 1000) * P + P, :])
                else:
                    nc.gpsimd.indirect_dma_start(
                        out=g[:], out_offset=None, in_=table[:],
                        in_offset=bass.IndirectOffsetOnAxis(ap=off_ap, axis=0))

                if first:
                    psA = psa.tile([P, F], dt.float32, tag="psA")
                    psB = psb.tile([F, P], dt.float32, tag="psB")

                oh = ohpool.tile([P, P], dt.float32, tag="oh")
                # S = (iota == dstloc) * ci_src
                nc.vector.tensor_scalar(
                    out=oh[:], in0=iota_f[:],
                    scalar1=dls_sb[:, mb:mb + 1],
                    scalar2=g[:, F2:F2 + 1],
                    op0=mybir.AluOpType.is_equal, op1=mybir.AluOpType.mult)

                nc.tensor.matmul(psA[:], lhsT=oh[:], rhs=g[:, 0:F],
                                 start=first, stop=False)
                nc.tensor.matmul(psA[:], lhsT=oh[:], rhs=g[:, F:F2],
                                 start=False, stop=False)
                tl = t % CHUNK
                nc.tensor.matmul(psB[:], lhsT=rfc[:, tl * F: tl * F + F],
                                 rhs=oh[:], start=first, stop=last)

                if last:
                    bt = btpool.tile([F, P], dt.float32, tag="bt")
                    nc.scalar.copy(bt[:], psB[:])
                    nc.tensor.matmul(psA[:], lhsT=bt[:], rhs=wrT_sb[:],
                                     start=False, stop=True)
                    o2 = opool.tile([P, F], dt.float32, tag="o2")
                    nc.vector.tensor_scalar_mul(o2[:], psA[:],
                                                cic_sb[:, k: k + 1])
                    rows = min(P, NPC - k * P)
                    nc.sync.dma_start(out=out[k * P: k * P + rows, :],
                                      in_=o2[:rows, :])

        if reps == 1:
            body()
        else:
            with tc.For_i(0, reps, 1) as iv:
                body(iv)

    nc.compile()
    return nc


# ------------------------------------------------------------------ driver

_CACHE = {}


def _get_program(meta, reps=1):
    key = (meta["N"], meta["F"], meta["NPC"], meta["K"], meta["NT"],
           meta["n_cores"], tuple(meta["nt_k"]), reps)
    if key not in _CACHE:
        _CACHE[key] = build_program(meta, reps=reps)
    return _CACHE[key]


def run(inputs, n_cores=N_CORES, trace=False, reps=1):
    in_maps, meta = host_prep(
        inputs["x"], inputs["weight"], inputs["w_review"],
        inputs["review_feat"], inputs["ci"], inputs["src"], inputs["dst"],
        n_cores)
    nc = _get_program(meta, reps=reps)
    res = run_bass_kernel_spmd(nc, in_maps, list(range(n_cores)), trace=trace)
    outp = np.concatenate([res.results[c]["out"] for c in range(n_cores)],
                          axis=0)
    return outp, res


def kernel(**inputs) -> np.ndarray:
    inputs = {k: np.asarray(v) for k, v in inputs.items()}
    last = None
    for attempt in range(3):
        try:
            outp, _ = run(inputs, n_cores=N_CORES)
            return outp
        except Exception as e:          # transient accelerator errors
            last = e
    raise last


# revision 10
# speedup vs baseline: 4.7351x; 1.0333x over previous
"""GCMCGraphConv forward on 8 trn2 NeuronCores (Bass/Tile).

reference:
    rf  = review_feat @ w_review.T                      [E, F]
    msg = (x[src] + weight[src] + rf) * ci[src]         [E, F]
    h   = segment_sum(msg, dst, N)                      [N, F]
    out = h * ci

Strategy (dst-owner sharding, edge-parallel within a core):
  - Core c owns nodes [c*NPC, (c+1)*NPC). Host routes every edge to the
    owner of its dst, groups the core's edges by 128-node block of dst,
    and pads each block's edge list to whole 128-edge tiles.  Tile
    counts per block are maxed across cores so all 8 cores run one SPMD
    program.
  - Device, per 128-edge tile:
      * one indirect DMA gathers 128 rows (528B) of a host-packed node
        table T = [x | weight | ci | pad] at the tile's src ids
      * DVE builds a ci-scaled one-hot:  S[e,n] = ci[src_e]*(dstloc_e==n)
      * PE accumulates into PSUM over the block's tiles:
          psA[n,0:64] += S^T @ x_rows ; psA += S^T @ w_rows
          psB[k,n]    += rfeat_tile(stationary) @ S       (= B^T)
  - Per block (once):  psA += B @ w_review^T
    via matmul(lhsT=B^T, rhs=w_review^T), then out_block = psA * ci[dst].
    (w_review commutes with the segment sum, so it is applied once per
     128-node block instead of once per edge.)

Host does index math / layout only (routing, padding, permutation,
concatenation); all float compute (gathers, messages, sums, matmuls,
scaling) runs on device.
"""

import os
import numpy as np
from contextlib import ExitStack

import concourse.bass as bass
import concourse.tile as tile
from concourse import bacc, mybir
from concourse.bass_utils import run_bass_kernel_spmd

P = 128
TBL_W = 132          # table row: 64 x | 64 w | 1 ci | 3 pad  (528B)
CHUNK = 16           # edge-tiles per rfeat DMA chunk
MBATCH = 512         # tiles per offs/dls metadata DMA
PAD_DL = 16000.0     # dstloc sentinel for pad edges -> one-hot column is 0

N_NODES = 150000
N_EDGES = 1250000
FEAT = 64
N_CORES = 8

# offsets handed to the indirect gather as a strided [P,1] column of the
# batched metadata tile (0) or copied to a dense [P,1] tile first (1).
OFF_COPY = bool(int(os.environ.get("GCMC_OFFCOPY", "0")))
# timing experiment: replace indirect gathers with contiguous loads of the
# same size (removes all Pool-engine work; output is wrong)
NO_GATHER = bool(int(os.environ.get("GCMC_NOGATHER", "0")))


# --------------------------------------------------------------- host prep

def host_prep(x, weight, w_review, review_feat, ci, src, dst, n_cores):
    """Route edges to dst-owner cores, build per-core DMA-friendly arrays.

    Index math and layout only -- no feature arithmetic happens here.
    """
    N, F = x.shape
    NPC = N // n_cores
    K = (NPC + P - 1) // P
    owner = dst // NPC

    per_core = []
    counts = np.zeros((n_cores, K), np.int64)
    for c in range(n_cores):
        sel = np.nonzero(owner == c)[0]
        blk = (dst[sel] - c * NPC) >> 7
        order = np.argsort(blk, kind="stable")
        per_core.append((sel[order], blk[order]))
        counts[c] = np.bincount(blk, minlength=K)

    nt_k = np.maximum(1, -(-counts.max(axis=0) // P))
    NT = int(nt_k.sum())
    NT16 = -(-NT // CHUNK) * CHUNK
    nt_k[K - 1] += NT16 - NT
    NT = NT16
    t0 = np.zeros(K + 1, np.int64)
    t0[1:] = np.cumsum(nt_k)

    table = np.zeros((N, TBL_W), np.float32)
    table[:, 0:F] = x
    table[:, F:2 * F] = weight
    table[:, 2 * F] = ci[:, 0]
    wrT = np.ascontiguousarray(w_review.T)               # [k, f]

    # slot -> rfeat DRAM row permutation (2 rows per 512B DMA line)
    slot_ids = np.arange(NT * P)
    t_of = slot_ids // P
    p_of = slot_ids % P
    tl = t_of % CHUNK
    rf_row = (((t_of // CHUNK) * 8 + tl // 2) * P + p_of) * 2 + (tl % 2)

    in_maps = []
    for c in range(n_cores):
        eids, blks = per_core[c]
        cnt = counts[c]
        base = np.concatenate([[0], np.cumsum(cnt)[:-1]])
        slotpos = t0[blks] * P + (np.arange(len(eids)) - base[blks])

        slots_src = np.zeros(NT * P, np.int32)
        slots_dl = np.full(NT * P, PAD_DL, np.float32)
        slots_src[slotpos] = src[eids]
        slots_dl[slotpos] = (dst[eids] - c * NPC - blks * P).astype(np.float32)

        rf = np.zeros((NT * P, F), np.float32)
        rf[rf_row[slotpos]] = review_feat[eids]

        nodes = c * NPC + np.arange(K * P)
        cic = np.zeros(K * P, np.float32)
        v = nodes < (c + 1) * NPC
        cic[v] = ci[nodes[v], 0]

        in_maps.append({
            "table": table,
            "wrT": wrT,
            "offs": np.ascontiguousarray(slots_src.reshape(NT, P).T),
            "dls": np.ascontiguousarray(slots_dl.reshape(NT, P).T),
            "rfs": rf,
            "cic": np.ascontiguousarray(cic.reshape(K, P).T),
        })

    meta = dict(N=N, F=F, NPC=NPC, K=K, NT=NT, n_cores=n_cores,
                nt_k=nt_k.tolist())
    return in_maps, meta


# ------------------------------------------------------------- bass program

def build_program(meta, reps=1):
    """Build the SPMD program.  reps>1 wraps the whole kernel in a hardware
    loop that re-executes it (idempotently) for wall-clock timing."""
    N = meta["N"]; F = meta["F"]; NPC = meta["NPC"]; K = meta["K"]
    NT = meta["NT"]; nt_k = meta["nt_k"]; n_cores = meta["n_cores"]
    F2 = 2 * F
    dt = mybir.dt

    t0 = np.zeros(K + 1, np.int64)
    t0[1:] = np.cumsum(nt_k)
    tile_block = np.repeat(np.arange(K), nt_k)

    nc = bacc.Bacc("TRN2", target_bir_lowering=False, debug=False,
                   enable_asserts=False, num_devices=n_cores)

    table = nc.dram_tensor("table", [N, TBL_W], dt.float32,
                           kind="ExternalInput").ap()
    wrT = nc.dram_tensor("wrT", [F, F], dt.float32, kind="ExternalInput").ap()
    offs = nc.dram_tensor("offs", [P, NT], dt.int32, kind="ExternalInput").ap()
    dls = nc.dram_tensor("dls", [P, NT], dt.float32,
                         kind="ExternalInput").ap()
    rfs = nc.dram_tensor("rfs", [NT * P, F], dt.float32,
                         kind="ExternalInput").ap()
    cic = nc.dram_tensor("cic", [P, K], dt.float32, kind="ExternalInput").ap()
    out = nc.dram_tensor("out", [NPC, F], dt.float32,
                         kind="ExternalOutput").ap()

    rf_view = rfs.rearrange("(c j p h) f -> c p j h f", j=8, p=P, h=2)

    with tile.TileContext(nc) as tc, ExitStack() as ctx:
        consts = ctx.enter_context(tc.tile_pool(name="consts", bufs=1))
        mpool = ctx.enter_context(tc.tile_pool(name="meta", bufs=2))
        gpool = ctx.enter_context(tc.tile_pool(name="gather", bufs=48))
        ofpool = ctx.enter_context(tc.tile_pool(name="ofp", bufs=24))
        rfpool = ctx.enter_context(tc.tile_pool(name="rfeat", bufs=6))
        ohpool = ctx.enter_context(tc.tile_pool(name="onehot", bufs=16))
        opool = ctx.enter_context(tc.tile_pool(name="outs", bufs=6))
        btpool = ctx.enter_context(tc.tile_pool(name="btile", bufs=3))
        psa = ctx.enter_context(tc.tile_pool(name="psa", bufs=4, space="PSUM"))
        psb = ctx.enter_context(tc.tile_pool(name="psb", bufs=4, space="PSUM"))

        iota_i = consts.tile([P, P], dt.int32, tag="iota_i")
        nc.gpsimd.iota(iota_i[:], pattern=[[1, P]], base=0,
                       channel_multiplier=0)
        iota_f = consts.tile([P, P], dt.float32, tag="iota_f")
        nc.vector.tensor_copy(iota_f[:], iota_i[:])
        wrT_sb = consts.tile([F, F], dt.float32, tag="wrT")
        nc.sync.dma_start(out=wrT_sb[:], in_=wrT[:])
        cic_sb = consts.tile([P, K], dt.float32, tag="cic")
        nc.sync.dma_start(out=cic_sb[:], in_=cic[:])

        def body(iv=None):
            offs_sb = dls_sb = rfc = None
            psA = psB = None
            for t in range(NT):
                k = int(tile_block[t])
                first = (t == t0[k])
                last = (t == t0[k + 1] - 1)
                mb = t % MBATCH
                if mb == 0:
                    w = min(MBATCH, NT - t)
                    offs_sb = mpool.tile([P, MBATCH], dt.int32, tag="offs")
                    nc.sync.dma_start(out=offs_sb[:, :w],
                                      in_=offs[:, t:t + w])
                    dls_sb = mpool.tile([P, MBATCH], dt.float32, tag="dls")
                    nc.sync.dma_start(out=dls_sb[:, :w], in_=dls[:, t:t + w])
                if t % CHUNK == 0:
                    rfc = rfpool.tile([P, CHUNK * F], dt.float32, tag="rfc")
                    nc.sync.dma_start(out=rfc[:], in_=rf_view[t // CHUNK])

                if OFF_COPY:
                    oc = ofpool.tile([P, 1], dt.int32, tag="oc")
                    nc.vector.tensor_copy(oc[:], offs_sb[:, mb:mb + 1])
                    off_ap = oc[:]
                else:
                    off_ap = offs_sb[:, mb:mb + 1]
                g = gpool.tile([P, TBL_W], dt.float32, tag="g")
                if NO_GATHER:
                    nc.sync.dma_start(
                        out=g[:], in_=table[(t % 1000) * P:(t % 1000) * P + P, :])
                else:
                    nc.gpsimd.indirect_dma_start(
                        out=g[:], out_offset=None, in_=table[:],
                        in_offset=bass.IndirectOffsetOnAxis(ap=off_ap, axis=0))

                if first:
                    psA = psa.tile([P, F], dt.float32, tag="psA")
                    psB = psb.tile([F, P], dt.float32, tag="psB")

                oh = ohpool.tile([P, P], dt.float32, tag="oh")
                # S = (iota == dstloc) * ci_src
                nc.vector.tensor_scalar(
                    out=oh[:], in0=iota_f[:],
                    scalar1=dls_sb[:, mb:mb + 1],
                    scalar2=g[:, F2:F2 + 1],
                    op0=mybir.AluOpType.is_equal, op1=mybir.AluOpType.mult)

                nc.tensor.matmul(psA[:], lhsT=oh[:], rhs=g[:, 0:F],
                                 start=first, stop=False)
                nc.tensor.matmul(psA[:], lhsT=oh[:], rhs=g[:, F:F2],
                                 start=False, stop=False)
                tl = t % CHUNK
                nc.tensor.matmul(psB[:], lhsT=rfc[:, tl * F: tl * F + F],
                                 rhs=oh[:], start=first, stop=last)

                if last:
                    bt = btpool.tile([F, P], dt.float32, tag="bt")
                    nc.scalar.copy(bt[:], psB[:])
                    nc.tensor.matmul(psA[:], lhsT=bt[:], rhs=wrT_sb[:],
                                     start=False, stop=True)
                    o2 = opool.tile([P, F], dt.float32, tag="o2")
                    nc.vector.tensor_scalar_mul(o2[:], psA[:],
                                                cic_sb[:, k: k + 1])
                    rows = min(P, NPC - k * P)
                    nc.sync.dma_start(out=out[k * P: k * P + rows, :],
                                      in_=o2[:rows, :])

        if reps == 1:
            body()
        else:
            with tc.For_i(0, reps, 1) as iv:
                body(iv)

    nc.compile()
    return nc


# ------------------------------------------------------------------ driver

_CACHE = {}


def _get_program(meta, reps=1):
    key = (meta["N"], meta["F"], meta["NPC"], meta["K"], meta["NT"],
           meta["n_cores"], tuple(meta["nt_k"]), reps)
    if key not in _CACHE:
        _CACHE[key] = build_program(meta, reps=reps)
    return _CACHE[key]


def run(inputs, n_cores=N_CORES, trace=False, reps=1):
    in_maps, meta = host_prep(
        inputs["x"], inputs["weight"], inputs["w_review"],
        inputs["review_feat"], inputs["ci"], inputs["src"], inputs["dst"],
        n_cores)
    nc = _get_program(meta, reps=reps)
    res = run_bass_kernel_spmd(nc, in_maps, list(range(n_cores)), trace=trace)
    outp = np.concatenate([res.results[c]["out"] for c in range(n_cores)],
                          axis=0)
    return outp, res


def kernel(**inputs) -> np.ndarray:
    inputs = {k: np.asarray(v) for k, v in inputs.items()}
    last = None
    for attempt in range(3):
        try:
            outp, _ = run(inputs, n_cores=N_CORES)
            return outp
        except Exception as e:          # transient accelerator errors
            last = e
    raise last
